# revision 16
# baseline (speedup 1.0000x reference)
"""Trainium2 Bass kernel for gated-adapter attention (Llama-Adapter style).

Sharding: 2 data-parallel groups of 4 cores (batch dim); within a group the 32
heads are tensor-parallel (8 heads/core).  Each core computes QKV + RoPE +
causal flash-style attention (transposed scores) + gated adapter cross
attention for its heads, AllGathers the per-head attention output across its
group of 4, then computes a column shard of the wo projection.  The host
reassembles the full [2, 2048, 4096] output from the 8 per-core shards.

All TensorEngine tensors are fp16 (1 cycle/row, ~0.05% rounding); PSUM
accumulation is fp32; softmax sums/reciprocals are fp32.
"""

import math
import os
import sys

import numpy as np

for _p in ("/opt/trn_rl_repo",):
    if os.path.isdir(_p) and _p not in sys.path:
        sys.path.insert(0, _p)

import ml_dtypes  # noqa: E402

import concourse.bass as bass  # noqa: E402
import concourse.mybir as mybir  # noqa: E402
import concourse.tile as tile  # noqa: E402
from concourse import bacc  # noqa: E402

FP16 = np.float16
F16 = mybir.dt.float16
F32 = mybir.dt.float32

DIM = 4096
S = 2048
B = 2
H = 32
HD = 128
ALEN = 10

NCORES = 8
CPG = 4          # cores per group (group = one batch element)
HPC = 8          # heads per core
OC = HPC * HD    # 1024 output dims per core for q/k/v and for wo columns

TCN = 17         # t-chunks of 128: 16 real + 1 adapter/pad chunk
TAUG = TCN * 128  # 2176
NM = DIM // 128   # 32 contraction chunks
QB = 4           # query blocks
QW = 512         # query block width
SCALE = 1.0 / math.sqrt(HD)

REPLICA_GROUPS = [[0, 1, 2, 3], [4, 5, 6, 7]]

Exp = mybir.ActivationFunctionType.Exp
Copy = mybir.ActivationFunctionType.Copy
MULT = None  # set after import in _alu()


def _alu():
    from concourse.alu_op_type import AluOpType
    return AluOpType


def build_graph():
    nc = bacc.Bacc(
        "TRN2",
        target_bir_lowering=False,
        debug=False,
        num_devices=NCORES,
    )

    # ---- external I/O ------------------------------------------------------
    xT = nc.dram_tensor("xT", [DIM, TAUG], F16, kind="ExternalInput")
    wqT = nc.dram_tensor("wqT", [DIM, OC], F16, kind="ExternalInput")
    wkT = nc.dram_tensor("wkT", [DIM, OC], F16, kind="ExternalInput")
    wvT = nc.dram_tensor("wvT", [DIM, OC], F16, kind="ExternalInput")
    woT = nc.dram_tensor("woT", [DIM, OC], F16, kind="ExternalInput")
    cosP = nc.dram_tensor("cosP", [S, 64], F16, kind="ExternalInput")
    sinP = nc.dram_tensor("sinP", [S, 64], F16, kind="ExternalInput")
    maskmul = nc.dram_tensor("maskmul", [QB, 4, 128, QW], F16, kind="ExternalInput")
    gates = nc.dram_tensor("gates", [16, HPC], F32, kind="ExternalInput")
    eye = nc.dram_tensor("eye", [128, 128], F16, kind="ExternalInput")
    out_ext = nc.dram_tensor("out", [S, OC], F32, kind="ExternalOutput")

    op = _alu()

    with tile.TileContext(nc) as tc:
        # persistent SBUF tensors -------------------------------------------
        with (
            tc.tile_pool(name="persist", bufs=1) as persist,
            tc.tile_pool(name="dram", bufs=1, space="DRAM") as dpool,
        ):
            qT = persist.tile([128, HPC * S], F16, tag="qT")     # [d, h*t]
            kT = persist.tile([128, HPC * S], F16, tag="kT")
            vsb = persist.tile([128, TCN * OC], F16, tag="vsb")  # [t, tc*o]
            akT = persist.tile([128, HPC * 16], F16, tag="akT")  # [d, h*16]
            avg = persist.tile([16, HPC * HD], F16, tag="avg")   # [a, h*d]
            ones = persist.tile([128, 1], F16, tag="ones")
            eyesb = persist.tile([128, 128], F16, tag="eyesb")
            gatesb = persist.tile([16, HPC], F32, tag="gatesb")
            cossb = persist.tile([128, 16 * 64], F16, tag="cossb")  # [t, tc*64]
            sinsb = persist.tile([128, 16 * 64], F16, tag="sinsb")

            negC = persist.tile([128, 1], F32, tag="negC")
            nc.gpsimd.memset(ones[:], 1.0)
            nc.gpsimd.memset(negC[:], -9.0)
            nc.sync.dma_start(eyesb[:], eye[:])
            nc.sync.dma_start(gatesb[:], gates[:])
            # cos/sin: [S, 64] -> [128, tc, 64]
            nc.sync.dma_start(
                cossb[:].rearrange("p (c d) -> p c d", c=16),
                cosP[:].rearrange("(c p) d -> p c d", p=128),
            )
            nc.sync.dma_start(
                sinsb[:].rearrange("p (c d) -> p c d", c=16),
                sinP[:].rearrange("(c p) d -> p c d", p=128),
            )

            agin = [dpool.tile([OC, QW], F16, tag=f"agin{q}", name=f"agin{q}")
                    for q in range(QB)]
            agout = [dpool.tile([CPG * OC, QW], F16, tag=f"agout{q}",
                                name=f"agout{q}")
                     for q in range(QB)]

            # ================= Phase 1: QKV + RoPE + transposes ============
            with (
                tc.tile_pool(name="wres", bufs=1) as wpool,
                tc.tile_pool(name="xin", bufs=6) as xpool,
                tc.tile_pool(name="asm", bufs=4) as apool,
                tc.tile_pool(name="rot", bufs=3) as rpool,
                tc.tile_pool(name="ps1", bufs=2, space="PSUM") as pspool,
                tc.tile_pool(name="pstr", bufs=2, space="PSUM") as ptpool,
            ):
                for proj, wsrc in (("q", wqT), ("k", wkT), ("v", wvT)):
                    ntc = 16 if proj == "q" else TCN
                    # resident weights for this projection: [128, 32*1024]
                    wres = wpool.tile([128, NM * OC], F16, tag="wres")
                    for m in range(NM):
                        nc.sync.dma_start(
                            wres[:, m * OC:(m + 1) * OC],
                            wsrc[m * 128:(m + 1) * 128, :],
                        )
                    for tch in range(ntc):
                        # x tiles re-DMA'd per projection (x streamed 3x)
                        ps = pspool.tile([128, OC], F32, tag="ps1")
                        for m in range(NM):
                            xm = xpool.tile([128, 128], F16, tag="xin")
                            nc.sync.dma_start(
                                xm[:],
                                xT[m * 128:(m + 1) * 128,
                                   tch * 128:(tch + 1) * 128],
                            )
                            for half in range(2):
                                nc.tensor.matmul(
                                    ps[:, half * 512:(half + 1) * 512],
                                    lhsT=xm[:],
                                    rhs=wres[:, m * OC + half * 512:
                                             m * OC + (half + 1) * 512],
                                    start=(m == 0),
                                    stop=(m == NM - 1),
                                )
                        if proj == "v":
                            # straight copy to resident v
                            nc.scalar.activation(
                                vsb[:, tch * OC:(tch + 1) * OC], ps[:], Copy)
                            continue
                        # assemble fp16 [t, o] tile
                        asmt = apool.tile([128, OC], F16, tag="asm")
                        nc.scalar.activation(asmt[:], ps[:], Copy)
                        if tch == 16:
                            # adapter chunk (k only): no rope; per-head
                            # transpose into akT
                            ptr = ptpool.tile([128, OC], F16, tag="pstr")
                            for h in range(HPC):
                                nc.tensor.transpose(
                                    ptr[:, h * 128:(h + 1) * 128],
                                    asmt[:, h * 128:(h + 1) * 128],
                                    eyesb[:],
                                )
                            nc.scalar.activation(
                                akT[:].rearrange("p (h a) -> p h a", h=HPC),
                                ptr[:].rearrange(
                                    "p (h t) -> p h t", h=HPC)[:, :, 0:16],
                                Copy,
                            )
                            continue
                        # ---- RoPE (deinterleaved head_dim layout) --------
                        # a' = a*cos - b*sin ; b' = a*sin + b*cos
                        a3 = asmt[:].rearrange(
                            "p (h d) -> p h d", h=HPC)[:, :, 0:64]
                        b3 = asmt[:].rearrange(
                            "p (h d) -> p h d", h=HPC)[:, :, 64:128]
                        cos1 = cossb[:].rearrange(
                            "p (c o d) -> p c o d", c=16, o=1)[:, tch]
                        sin1 = sinsb[:].rearrange(
                            "p (c o d) -> p c o d", c=16, o=1)[:, tch]
                        cos3, _ = bass.broadcast_tensor_aps(cos1, a3)
                        sin3, _ = bass.broadcast_tensor_aps(sin1, a3)
                        rot = rpool.tile([128, OC], F16, tag="rot")
                        ra = rot[:].rearrange(
                            "p (h d) -> p h d", h=HPC)[:, :, 0:64]
                        rb = rot[:].rearrange(
                            "p (h d) -> p h d", h=HPC)[:, :, 64:128]
                        t1 = rpool.tile([128, HPC * 64], F16, tag="ropetmp")
                        t13 = t1[:].rearrange("p (h d) -> p h d", h=HPC)
                        t2 = rpool.tile([128, HPC * 64], F16, tag="ropetmp2")
                        t23 = t2[:].rearrange("p (h d) -> p h d", h=HPC)
                        nc.vector.tensor_tensor(t13, a3, cos3, op.mult)
                        nc.vector.tensor_tensor(t23, b3, sin3, op.mult)
                        nc.vector.tensor_tensor(ra, t13, t23, op.subtract)
                        nc.vector.tensor_tensor(t13, a3, sin3, op.mult)
                        nc.vector.tensor_tensor(t23, b3, cos3, op.mult)
                        nc.vector.tensor_tensor(rb, t13, t23, op.add)
                        # ---- transpose per head into qT/kT ----------------
                        dst = qT if proj == "q" else kT
                        ptr = ptpool.tile([128, OC], F16, tag="pstr")
                        for h in range(HPC):
                            nc.tensor.transpose(
                                ptr[:, h * 128:(h + 1) * 128],
                                rot[:, h * 128:(h + 1) * 128],
                                eyesb[:],
                            )
                        nc.scalar.activation(
                            dst[:].rearrange(
                                "p (h t) -> p h t",
                                h=HPC)[:, :, tch * 128:(tch + 1) * 128],
                            ptr[:].rearrange("p (h d) -> p h d", h=HPC),
                            Copy,
                        )

            # gated adapter V: avg[a, h*d] = gate_h * v[a, h*d]  (a = 0..9)
            for h in range(HPC):
                nc.vector.tensor_scalar(
                    avg[0:10, h * HD:(h + 1) * HD],
                    vsb[0:10, 16 * OC + h * HD: 16 * OC + (h + 1) * HD],
                    gatesb[0:10, h:h + 1],
                    None,
                    op.mult,
                )

            # ================= Phase 2: attention ==========================
            with (
                tc.tile_pool(name="probs", bufs=20) as prpool,
                tc.tile_pool(name="mask", bufs=5) as mkpool,
                tc.tile_pool(name="small", bufs=4) as smpool,
                tc.tile_pool(name="bcast", bufs=4) as bcpool,
                tc.tile_pool(name="ctmp", bufs=4) as ctpool,
                tc.tile_pool(name="psc", bufs=2, space="PSUM") as pscp,
                tc.tile_pool(name="ppv", bufs=1, space="PSUM") as ppvp,
                tc.tile_pool(name="papv", bufs=1, space="PSUM") as papvp,
                tc.tile_pool(name="pasc", bufs=1, space="PSUM") as pascp,
                tc.tile_pool(name="psumM", bufs=1, space="PSUM") as psmp,
                tc.tile_pool(name="psumA", bufs=1, space="PSUM") as psap,
            ):
                for qb in range(QB):
                    kk = (qb + 1) * 4  # causal: k chunks 0..kk-1
                    mtiles = []
                    for dk in range(4):
                        mt = mkpool.tile([128, QW], F16, tag="mask")
                        nc.sync.dma_start(mt[:], maskmul[qb, dk])
                        mtiles.append(mt)
                    for h in range(HPC):
                        q_ap = qT[:, h * S + qb * QW: h * S + (qb + 1) * QW]
                        probs = []
                        for kc in range(kk):
                            sc = pscp.tile([128, QW], F32, tag="sc")
                            nc.tensor.matmul(
                                sc[:],
                                lhsT=kT[:, h * S + kc * 128:
                                        h * S + (kc + 1) * 128],
                                rhs=q_ap,
                                start=True, stop=True,
                            )
                            pb = prpool.tile([128, QW], F16, tag="probs")
                            # -7 bias: softmax shift-invariance keeps fp16
                            # exp outputs in range (max score*scale ~ 14.6)
                            nc.scalar.activation(pb[:], sc[:], Exp,
                                                 bias=negC[:, 0:1],
                                                 scale=SCALE)
                            if kc >= qb * 4:
                                nc.vector.tensor_tensor(
                                    pb[:], pb[:], mtiles[kc - qb * 4][:],
                                    op.mult)
                            probs.append(pb)
                        sumM = psmp.tile([1, QW], F32, tag="sumM")
                        for i, pb in enumerate(probs):
                            nc.tensor.matmul(
                                sumM[:], lhsT=ones[:, 0:1], rhs=pb[:],
                                start=(i == 0), stop=(i == kk - 1),
                            )
                        pv = ppvp.tile([128, QW], F32, tag="pv")
                        for i, pb in enumerate(probs):
                            nc.tensor.matmul(
                                pv[:],
                                lhsT=vsb[:, i * OC + h * HD:
                                         i * OC + (h + 1) * HD],
                                rhs=pb[:],
                                start=(i == 0), stop=(i == kk - 1),
                            )
                        # adapter
                        asc = pascp.tile([10, QW], F32, tag="asc")
                        nc.tensor.matmul(
                            asc[:], lhsT=akT[:, h * 16:h * 16 + 10], rhs=q_ap,
                            start=True, stop=True)
                        apb = smpool.tile([10, QW], F16, tag="aprobs")
                        nc.scalar.activation(apb[:], asc[:], Exp,
                                             bias=negC[0:10, 0:1],
                                             scale=SCALE)
                        sumA = psap.tile([1, QW], F32, tag="sumA")
                        nc.tensor.matmul(
                            sumA[:], lhsT=ones[0:10, 0:1], rhs=apb[:],
                            start=True, stop=True)
                        apv = papvp.tile([128, QW], F32, tag="apv")
                        nc.tensor.matmul(
                            apv[:], lhsT=avg[0:10, h * HD:(h + 1) * HD],
                            rhs=apb[:], start=True, stop=True)
                        # normalize + combine
                        recM = smpool.tile([1, QW], F32, tag="recM")
                        nc.vector.reciprocal_approx_fast(recM[:], sumM[:])
                        recA = smpool.tile([1, QW], F32, tag="recA")
                        nc.vector.reciprocal_approx_fast(recA[:], sumA[:])
                        bcM = bcpool.tile([128, QW], F32, tag="bcM")
                        nc.gpsimd.partition_broadcast(bcM[:], recM[:])
                        bcA = bcpool.tile([128, QW], F32, tag="bcA")
                        nc.gpsimd.partition_broadcast(bcA[:], recA[:])
                        c1 = ctpool.tile([128, QW], F32, tag="c1")
                        nc.vector.tensor_tensor(c1[:], pv[:], bcM[:], op.mult)
                        c2 = ctpool.tile([128, QW], F32, tag="c2")
                        nc.vector.tensor_tensor(c2[:], apv[:], bcA[:], op.mult)
                        c3 = ctpool.tile([128, QW], F16, tag="c3")
                        nc.vector.tensor_tensor(c3[:], c1[:], c2[:], op.add)
                        nc.sync.dma_start(
                            agin[qb][h * 128:(h + 1) * 128, :], c3[:])
                    # AllGather this query block across the group
                    nc.gpsimd.collective_compute(
                        "AllGather",
                        op.bypass,
                        replica_groups=REPLICA_GROUPS,
                        ins=[agin[qb][:].opt()],
                        outs=[agout[qb][:].opt()],
                    )

            # ================= Phase 3: wo projection ======================
            with (
                tc.tile_pool(name="w2", bufs=1) as w2pool,
                tc.tile_pool(name="agsb", bufs=36) as agpool,
                tc.tile_pool(name="ost", bufs=3) as ostpool,
                tc.tile_pool(name="pwo", bufs=2, space="PSUM") as pwop,
            ):
                for jh in range(2):  # output-column halves of wo
                    w2 = w2pool.tile([128, NM * 512], F16, tag="w2")
                    for m in range(NM):
                        nc.sync.dma_start(
                            w2[:, m * 512:(m + 1) * 512],
                            woT[m * 128:(m + 1) * 128,
                                jh * 512:(jh + 1) * 512])
                    for qb in range(QB):
                        ag = []
                        for i in range(NM):
                            a = agpool.tile([128, QW], F16, tag="agsb",
                                            name=f"ag{jh}_{qb}_{i}")
                            nc.sync.dma_start(
                                a[:], agout[qb][i * 128:(i + 1) * 128, :])
                            ag.append(a)
                        for tsub in range(4):
                            ps = pwop.tile([128, 512], F32, tag="pwo")
                            for i in range(NM):
                                nc.tensor.matmul(
                                    ps[:],
                                    lhsT=ag[i][:, tsub * 128:(tsub + 1) * 128],
                                    rhs=w2[:, i * 512:(i + 1) * 512],
                                    start=(i == 0), stop=(i == NM - 1),
                                )
                            st = ostpool.tile([128, 512], F32, tag="ost")
                            nc.scalar.activation(st[:], ps[:], Copy)
                            r0 = qb * QW + tsub * 128
                            nc.sync.dma_start(
                                out_ext[r0:r0 + 128,
                                        jh * 512:(jh + 1) * 512], st[:])

    nc.compile()
    return nc


# ---------------------------------------------------------------------------
# host-side input prep + execution
# ---------------------------------------------------------------------------

_DEINT = np.concatenate([np.arange(0, 128, 2), np.arange(1, 128, 2)])


def _prep_inputs(x, adapter, wq, wk, wv, wo, gate, freqs_cos, freqs_sin, mask):
    """Build the per-core input maps."""
    perm = np.concatenate([h * HD + _DEINT for h in range(H)])  # deinterleave
    wqp = wq[perm, :]  # permute output dims of wq/wk for rope layout
    wkp = wk[perm, :]

    in_maps = []
    for c in range(NCORES):
        g, ci = divmod(c, CPG)
        osl = slice(ci * OC, (ci + 1) * OC)
        xT = np.zeros((DIM, TAUG), FP16)
        xT[:, :S] = x[g].T.astype(FP16)
        xT[:, S:S + ALEN] = adapter[0].T.astype(FP16)
        mm = np.empty((QB, 4, 128, QW), FP16)
        for qb in range(QB):
            q0 = qb * QW
            for dk in range(4):
                k0 = q0 + dk * 128
                mm[qb, dk] = np.exp(
                    mask[0, 0, q0:q0 + QW, k0:k0 + 128]).T.astype(FP16)
        gates = np.zeros((16, HPC), np.float32)
        gates[:, :] = gate[0, ci * HPC:(ci + 1) * HPC, 0, 0][None, :]
        in_maps.append({
            "xT": xT,
            "wqT": np.ascontiguousarray(wqp[osl].T).astype(FP16),
            "wkT": np.ascontiguousarray(wkp[osl].T).astype(FP16),
            "wvT": np.ascontiguousarray(wv[osl].T).astype(FP16),
            "woT": np.ascontiguousarray(wo[osl].T).astype(FP16),
            "cosP": freqs_cos.astype(FP16),
            "sinP": freqs_sin.astype(FP16),
            "maskmul": mm,
            "gates": gates,
            "eye": np.eye(128, dtype=FP16),
        })
    return in_maps


_NC_CACHE = {}
TRACE = bool(int(os.environ.get("BASS_KERNEL_TRACE", "0")))
LAST_EXEC_NS = None
LAST_RESULTS = None


def kernel(x, adapter, wq, wk, wv, wo, gate, freqs_cos, freqs_sin, mask,
           start_pos=0, **_unused):
    global LAST_EXEC_NS, LAST_RESULTS
    from concourse.bass_utils import run_bass_kernel_spmd

    to_np = lambda a: np.asarray(a)
    x, adapter, wq, wk, wv, wo = map(to_np, (x, adapter, wq, wk, wv, wo))
    gate, freqs_cos, freqs_sin, mask = map(
        to_np, (gate, freqs_cos, freqs_sin, mask))

    if "nc" not in _NC_CACHE:
        _NC_CACHE["nc"] = build_graph()
    nc = _NC_CACHE["nc"]

    in_maps = _prep_inputs(x, adapter, wq, wk, wv, wo, gate,
                           freqs_cos, freqs_sin, mask)
    res = run_bass_kernel_spmd(
        nc, in_maps, core_ids=list(range(NCORES)), trace=TRACE)
    LAST_EXEC_NS = res.exec_time_ns
    LAST_RESULTS = res
    out = np.empty((B, S, DIM), np.float32)
    for c in range(NCORES):
        g, ci = divmod(c, CPG)
        out[g, :, ci * OC:(ci + 1) * OC] = res.results[c]["out"]
    return out


# revision 18
# speedup vs baseline: 1.0136x; 1.0136x over previous
"""Trainium2 Bass kernel for gated-adapter attention (Llama-Adapter style).

Sharding: 2 data-parallel groups of 4 cores (batch dim); within a group the 32
heads are tensor-parallel (8 heads/core).  Each core computes QKV + RoPE +
causal flash-style attention (transposed scores) + gated adapter cross
attention for its heads, AllGathers the per-head attention output across its
group of 4, then computes a column shard of the wo projection.  The host
reassembles the full [2, 2048, 4096] output from the 8 per-core shards.

All TensorEngine tensors are fp16 (1 cycle/row, ~0.05% rounding); PSUM
accumulation is fp32; softmax sums/reciprocals are fp32.
"""

import math
import os
import sys

import numpy as np

for _p in ("/opt/trn_rl_repo",):
    if os.path.isdir(_p) and _p not in sys.path:
        sys.path.insert(0, _p)

import ml_dtypes  # noqa: E402

import concourse.bass as bass  # noqa: E402
import concourse.mybir as mybir  # noqa: E402
import concourse.tile as tile  # noqa: E402
from concourse import bacc  # noqa: E402

FP16 = np.float16
F16 = mybir.dt.float16
F32 = mybir.dt.float32

DIM = 4096
S = 2048
B = 2
H = 32
HD = 128
ALEN = 10

NCORES = 8
CPG = 4          # cores per group (group = one batch element)
HPC = 8          # heads per core
OC = HPC * HD    # 1024 output dims per core for q/k/v and for wo columns

TCN = 17         # t-chunks of 128: 16 real + 1 adapter/pad chunk
TAUG = TCN * 128  # 2176
NM = DIM // 128   # 32 contraction chunks
QB = 4           # query blocks
QW = 512         # query block width
SCALE = 1.0 / math.sqrt(HD)

REPLICA_GROUPS = [[0, 1, 2, 3], [4, 5, 6, 7]]

Exp = mybir.ActivationFunctionType.Exp
Copy = mybir.ActivationFunctionType.Copy
MULT = None  # set after import in _alu()


def _alu():
    from concourse.alu_op_type import AluOpType
    return AluOpType


def build_graph():
    nc = bacc.Bacc(
        "TRN2",
        target_bir_lowering=False,
        debug=False,
        num_devices=NCORES,
    )

    # ---- external I/O ------------------------------------------------------
    xT = nc.dram_tensor("xT", [DIM, TAUG], F16, kind="ExternalInput")
    wqT = nc.dram_tensor("wqT", [DIM, OC], F16, kind="ExternalInput")
    wkT = nc.dram_tensor("wkT", [DIM, OC], F16, kind="ExternalInput")
    wvT = nc.dram_tensor("wvT", [DIM, OC], F16, kind="ExternalInput")
    woT = nc.dram_tensor("woT", [DIM, OC], F16, kind="ExternalInput")
    cosP = nc.dram_tensor("cosP", [S, 64], F16, kind="ExternalInput")
    sinP = nc.dram_tensor("sinP", [S, 64], F16, kind="ExternalInput")
    maskmul = nc.dram_tensor("maskmul", [QB, 4, 128, QW], F16, kind="ExternalInput")
    gates = nc.dram_tensor("gates", [16, HPC], F32, kind="ExternalInput")
    eye = nc.dram_tensor("eye", [128, 128], F16, kind="ExternalInput")
    out_ext = nc.dram_tensor("out", [S, OC], F32, kind="ExternalOutput")

    op = _alu()

    with tile.TileContext(nc) as tc:
        # persistent SBUF tensors -------------------------------------------
        with (
            tc.tile_pool(name="persist", bufs=1) as persist,
            tc.tile_pool(name="dram", bufs=1, space="DRAM") as dpool,
        ):
            qT = persist.tile([128, HPC * S], F16, tag="qT")     # [d, h*t]
            kT = persist.tile([128, HPC * S], F16, tag="kT")
            vsb = persist.tile([128, TCN * OC], F16, tag="vsb")  # [t, tc*o]
            akT = persist.tile([128, HPC * 16], F16, tag="akT")  # [d, h*16]
            avg = persist.tile([16, HPC * HD], F16, tag="avg")   # [a, h*d]
            ones = persist.tile([128, 1], F16, tag="ones")
            eyesb = persist.tile([128, 128], F16, tag="eyesb")
            gatesb = persist.tile([16, HPC], F32, tag="gatesb")
            cossb = persist.tile([128, 16 * 64], F16, tag="cossb")  # [t, tc*64]
            sinsb = persist.tile([128, 16 * 64], F16, tag="sinsb")

            negC = persist.tile([128, 1], F32, tag="negC")
            nc.gpsimd.memset(ones[:], 1.0)
            nc.gpsimd.memset(negC[:], -9.0)
            nc.sync.dma_start(eyesb[:], eye[:])
            nc.sync.dma_start(gatesb[:], gates[:])
            # cos/sin: [S, 64] -> [128, tc, 64]
            nc.sync.dma_start(
                cossb[:].rearrange("p (c d) -> p c d", c=16),
                cosP[:].rearrange("(c p) d -> p c d", p=128),
            )
            nc.sync.dma_start(
                sinsb[:].rearrange("p (c d) -> p c d", c=16),
                sinP[:].rearrange("(c p) d -> p c d", p=128),
            )

            agin = [dpool.tile([OC, QW], F16, tag=f"agin{q}", name=f"agin{q}")
                    for q in range(QB)]
            agout = [dpool.tile([CPG * OC, QW], F16, tag=f"agout{q}",
                                name=f"agout{q}")
                     for q in range(QB)]

            # ================= Phase 1: QKV + RoPE + transposes ============
            with (
                tc.tile_pool(name="wres", bufs=36) as wpool,
                tc.tile_pool(name="xin", bufs=6) as xpool,
                tc.tile_pool(name="asm", bufs=4) as apool,
                tc.tile_pool(name="rot", bufs=3) as rpool,
                tc.tile_pool(name="ps1", bufs=2, space="PSUM") as pspool,
                tc.tile_pool(name="pstr", bufs=2, space="PSUM") as ptpool,
            ):
                for proj, wsrc in (("q", wqT), ("k", wkT), ("v", wvT)):
                    ntc = 16 if proj == "q" else TCN
                    # per-m-chunk weight tiles: slots recycle across
                    # projections so the next projection's weights stream in
                    # while this one finishes (no PE stall at the boundary)
                    wres = []
                    for m in range(NM):
                        wt = wpool.tile([128, OC], F16, tag="wres",
                                        name=f"w{proj}{m}")
                        nc.sync.dma_start(
                            wt[:], wsrc[m * 128:(m + 1) * 128, :])
                        wres.append(wt)
                    for tch in range(ntc):
                        # x tiles re-DMA'd per projection (x streamed 3x)
                        ps = pspool.tile([128, OC], F32, tag="ps1")
                        for m in range(NM):
                            xm = xpool.tile([128, 128], F16, tag="xin")
                            nc.sync.dma_start(
                                xm[:],
                                xT[m * 128:(m + 1) * 128,
                                   tch * 128:(tch + 1) * 128],
                            )
                            for half in range(2):
                                nc.tensor.matmul(
                                    ps[:, half * 512:(half + 1) * 512],
                                    lhsT=xm[:],
                                    rhs=wres[m][:, half * 512:
                                                 (half + 1) * 512],
                                    start=(m == 0),
                                    stop=(m == NM - 1),
                                )
                        if proj == "v":
                            # straight copy to resident v
                            nc.scalar.activation(
                                vsb[:, tch * OC:(tch + 1) * OC], ps[:], Copy)
                            continue
                        # assemble fp16 [t, o] tile
                        asmt = apool.tile([128, OC], F16, tag="asm")
                        nc.scalar.activation(asmt[:], ps[:], Copy)
                        if tch == 16:
                            # adapter chunk (k only): no rope; per-head
                            # transpose into akT
                            ptr = ptpool.tile([128, OC], F16, tag="pstr")
                            for h in range(HPC):
                                nc.tensor.transpose(
                                    ptr[:, h * 128:(h + 1) * 128],
                                    asmt[:, h * 128:(h + 1) * 128],
                                    eyesb[:],
                                )
                            nc.scalar.activation(
                                akT[:].rearrange("p (h a) -> p h a", h=HPC),
                                ptr[:].rearrange(
                                    "p (h t) -> p h t", h=HPC)[:, :, 0:16],
                                Copy,
                            )
                            continue
                        # ---- RoPE (deinterleaved head_dim layout) --------
                        # a' = a*cos - b*sin ; b' = a*sin + b*cos
                        a3 = asmt[:].rearrange(
                            "p (h d) -> p h d", h=HPC)[:, :, 0:64]
                        b3 = asmt[:].rearrange(
                            "p (h d) -> p h d", h=HPC)[:, :, 64:128]
                        cos1 = cossb[:].rearrange(
                            "p (c o d) -> p c o d", c=16, o=1)[:, tch]
                        sin1 = sinsb[:].rearrange(
                            "p (c o d) -> p c o d", c=16, o=1)[:, tch]
                        cos3, _ = bass.broadcast_tensor_aps(cos1, a3)
                        sin3, _ = bass.broadcast_tensor_aps(sin1, a3)
                        rot = rpool.tile([128, OC], F16, tag="rot")
                        ra = rot[:].rearrange(
                            "p (h d) -> p h d", h=HPC)[:, :, 0:64]
                        rb = rot[:].rearrange(
                            "p (h d) -> p h d", h=HPC)[:, :, 64:128]
                        t1 = rpool.tile([128, HPC * 64], F16, tag="ropetmp")
                        t13 = t1[:].rearrange("p (h d) -> p h d", h=HPC)
                        t2 = rpool.tile([128, HPC * 64], F16, tag="ropetmp2")
                        t23 = t2[:].rearrange("p (h d) -> p h d", h=HPC)
                        nc.vector.tensor_tensor(t13, a3, cos3, op.mult)
                        nc.vector.tensor_tensor(t23, b3, sin3, op.mult)
                        nc.vector.tensor_tensor(ra, t13, t23, op.subtract)
                        nc.vector.tensor_tensor(t13, a3, sin3, op.mult)
                        nc.vector.tensor_tensor(t23, b3, cos3, op.mult)
                        nc.vector.tensor_tensor(rb, t13, t23, op.add)
                        # ---- transpose per head into qT/kT ----------------
                        dst = qT if proj == "q" else kT
                        ptr = ptpool.tile([128, OC], F16, tag="pstr")
                        for h in range(HPC):
                            nc.tensor.transpose(
                                ptr[:, h * 128:(h + 1) * 128],
                                rot[:, h * 128:(h + 1) * 128],
                                eyesb[:],
                            )
                        nc.scalar.activation(
                            dst[:].rearrange(
                                "p (h t) -> p h t",
                                h=HPC)[:, :, tch * 128:(tch + 1) * 128],
                            ptr[:].rearrange("p (h d) -> p h d", h=HPC),
                            Copy,
                        )

            # gated adapter V: avg[a, h*d] = gate_h * v[a, h*d]  (a = 0..9)
            for h in range(HPC):
                nc.vector.tensor_scalar(
                    avg[0:10, h * HD:(h + 1) * HD],
                    vsb[0:10, 16 * OC + h * HD: 16 * OC + (h + 1) * HD],
                    gatesb[0:10, h:h + 1],
                    None,
                    op.mult,
                )

            # ================= Phase 2: attention ==========================
            with (
                tc.tile_pool(name="probs", bufs=20) as prpool,
                tc.tile_pool(name="mask", bufs=5) as mkpool,
                tc.tile_pool(name="small", bufs=4) as smpool,
                tc.tile_pool(name="bcast", bufs=4) as bcpool,
                tc.tile_pool(name="ctmp", bufs=4) as ctpool,
                tc.tile_pool(name="psc", bufs=3, space="PSUM") as pscp,
                tc.tile_pool(name="ppv", bufs=3, space="PSUM") as ppvp,
                tc.tile_pool(name="psumM", bufs=1, space="PSUM") as psmp,
                tc.tile_pool(name="psumA", bufs=1, space="PSUM") as psap,
            ):
                for qb in range(QB - 1, -1, -1):
                    kk = (qb + 1) * 4  # causal: k chunks 0..kk-1
                    mtiles = []
                    for dk in range(4):
                        mt = mkpool.tile([128, QW], F16, tag="mask")
                        nc.sync.dma_start(mt[:], maskmul[qb, dk])
                        mtiles.append(mt)
                    for h in range(HPC):
                        q_ap = qT[:, h * S + qb * QW: h * S + (qb + 1) * QW]
                        probs = []
                        for kc in range(kk):
                            sc = pscp.tile([128, QW], F32, tag="sc")
                            nc.tensor.matmul(
                                sc[:],
                                lhsT=kT[:, h * S + kc * 128:
                                        h * S + (kc + 1) * 128],
                                rhs=q_ap,
                                start=True, stop=True,
                            )
                            pb = prpool.tile([128, QW], F16, tag="probs")
                            # -7 bias: softmax shift-invariance keeps fp16
                            # exp outputs in range (max score*scale ~ 14.6)
                            nc.scalar.activation(pb[:], sc[:], Exp,
                                                 bias=negC[:, 0:1],
                                                 scale=SCALE)
                            if kc >= qb * 4:
                                nc.vector.tensor_tensor(
                                    pb[:], pb[:], mtiles[kc - qb * 4][:],
                                    op.mult)
                            probs.append(pb)
                        sumM = psmp.tile([1, QW], F32, tag="sumM")
                        for i, pb in enumerate(probs):
                            nc.tensor.matmul(
                                sumM[:], lhsT=ones[:, 0:1], rhs=pb[:],
                                start=(i == 0), stop=(i == kk - 1),
                            )
                        pv = ppvp.tile([128, QW], F32, tag="pv")
                        for i, pb in enumerate(probs):
                            nc.tensor.matmul(
                                pv[:],
                                lhsT=vsb[:, i * OC + h * HD:
                                         i * OC + (h + 1) * HD],
                                rhs=pb[:],
                                start=(i == 0), stop=(i == kk - 1),
                            )
                        # adapter
                        asc = pscp.tile([10, QW], F32, tag="sc",
                                        name=f"asc{qb}_{h}")
                        nc.tensor.matmul(
                            asc[:], lhsT=akT[:, h * 16:h * 16 + 10], rhs=q_ap,
                            start=True, stop=True)
                        apb = smpool.tile([10, QW], F16, tag="aprobs")
                        nc.scalar.activation(apb[:], asc[:], Exp,
                                             bias=negC[0:10, 0:1],
                                             scale=SCALE)
                        sumA = psap.tile([1, QW], F32, tag="sumA")
                        nc.tensor.matmul(
                            sumA[:], lhsT=ones[0:10, 0:1], rhs=apb[:],
                            start=True, stop=True)
                        apv = ppvp.tile([128, QW], F32, tag="pv",
                                        name=f"apv{qb}_{h}")
                        nc.tensor.matmul(
                            apv[:], lhsT=avg[0:10, h * HD:(h + 1) * HD],
                            rhs=apb[:], start=True, stop=True)
                        # normalize + combine
                        recM = smpool.tile([1, QW], F32, tag="recM")
                        nc.vector.reciprocal_approx_fast(recM[:], sumM[:])
                        recA = smpool.tile([1, QW], F32, tag="recA")
                        nc.vector.reciprocal_approx_fast(recA[:], sumA[:])
                        bcM = bcpool.tile([128, QW], F32, tag="bcM")
                        nc.gpsimd.partition_broadcast(bcM[:], recM[:])
                        bcA = bcpool.tile([128, QW], F32, tag="bcA")
                        nc.gpsimd.partition_broadcast(bcA[:], recA[:])
                        c1 = ctpool.tile([128, QW], F32, tag="c1")
                        nc.vector.tensor_tensor(c1[:], pv[:], bcM[:], op.mult)
                        c2 = ctpool.tile([128, QW], F32, tag="c2")
                        nc.vector.tensor_tensor(c2[:], apv[:], bcA[:], op.mult)
                        c3 = ctpool.tile([128, QW], F16, tag="c3")
                        nc.vector.tensor_tensor(c3[:], c1[:], c2[:], op.add)
                        nc.sync.dma_start(
                            agin[qb][h * 128:(h + 1) * 128, :], c3[:])
                    # AllGather this query block across the group
                    nc.gpsimd.collective_compute(
                        "AllGather",
                        op.bypass,
                        replica_groups=REPLICA_GROUPS,
                        ins=[agin[qb][:].opt()],
                        outs=[agout[qb][:].opt()],
                    )

            # ================= Phase 3: wo projection ======================
            with (
                tc.tile_pool(name="w2", bufs=1) as w2pool,
                tc.tile_pool(name="agsb", bufs=36) as agpool,
                tc.tile_pool(name="ost", bufs=3) as ostpool,
                tc.tile_pool(name="pwo", bufs=2, space="PSUM") as pwop,
            ):
                for jh in range(2):  # output-column halves of wo
                    w2 = w2pool.tile([128, NM * 512], F16, tag="w2")
                    for m in range(NM):
                        nc.sync.dma_start(
                            w2[:, m * 512:(m + 1) * 512],
                            woT[m * 128:(m + 1) * 128,
                                jh * 512:(jh + 1) * 512])
                    for qb in range(QB - 1, -1, -1):
                        ag = []
                        for i in range(NM):
                            a = agpool.tile([128, QW], F16, tag="agsb",
                                            name=f"ag{jh}_{qb}_{i}")
                            nc.sync.dma_start(
                                a[:], agout[qb][i * 128:(i + 1) * 128, :])
                            ag.append(a)
                        for tsub in range(4):
                            ps = pwop.tile([128, 512], F32, tag="pwo")
                            for i in range(NM):
                                nc.tensor.matmul(
                                    ps[:],
                                    lhsT=ag[i][:, tsub * 128:(tsub + 1) * 128],
                                    rhs=w2[:, i * 512:(i + 1) * 512],
                                    start=(i == 0), stop=(i == NM - 1),
                                )
                            st = ostpool.tile([128, 512], F32, tag="ost")
                            nc.scalar.activation(st[:], ps[:], Copy)
                            r0 = qb * QW + tsub * 128
                            nc.sync.dma_start(
                                out_ext[r0:r0 + 128,
                                        jh * 512:(jh + 1) * 512], st[:])

    nc.compile()
    return nc


# ---------------------------------------------------------------------------
# host-side input prep + execution
# ---------------------------------------------------------------------------

_DEINT = np.concatenate([np.arange(0, 128, 2), np.arange(1, 128, 2)])


def _prep_inputs(x, adapter, wq, wk, wv, wo, gate, freqs_cos, freqs_sin, mask):
    """Build the per-core input maps."""
    perm = np.concatenate([h * HD + _DEINT for h in range(H)])  # deinterleave
    wqp = wq[perm, :]  # permute output dims of wq/wk for rope layout
    wkp = wk[perm, :]

    in_maps = []
    for c in range(NCORES):
        g, ci = divmod(c, CPG)
        osl = slice(ci * OC, (ci + 1) * OC)
        xT = np.zeros((DIM, TAUG), FP16)
        xT[:, :S] = x[g].T.astype(FP16)
        xT[:, S:S + ALEN] = adapter[0].T.astype(FP16)
        mm = np.empty((QB, 4, 128, QW), FP16)
        for qb in range(QB):
            q0 = qb * QW
            for dk in range(4):
                k0 = q0 + dk * 128
                mm[qb, dk] = np.exp(
                    mask[0, 0, q0:q0 + QW, k0:k0 + 128]).T.astype(FP16)
        gates = np.zeros((16, HPC), np.float32)
        gates[:, :] = gate[0, ci * HPC:(ci + 1) * HPC, 0, 0][None, :]
        in_maps.append({
            "xT": xT,
            "wqT": np.ascontiguousarray(wqp[osl].T).astype(FP16),
            "wkT": np.ascontiguousarray(wkp[osl].T).astype(FP16),
            "wvT": np.ascontiguousarray(wv[osl].T).astype(FP16),
            "woT": np.ascontiguousarray(wo[osl].T).astype(FP16),
            "cosP": freqs_cos.astype(FP16),
            "sinP": freqs_sin.astype(FP16),
            "maskmul": mm,
            "gates": gates,
            "eye": np.eye(128, dtype=FP16),
        })
    return in_maps


_NC_CACHE = {}
TRACE = bool(int(os.environ.get("BASS_KERNEL_TRACE", "0")))
LAST_EXEC_NS = None
LAST_RESULTS = None


def kernel(x, adapter, wq, wk, wv, wo, gate, freqs_cos, freqs_sin, mask,
           start_pos=0, **_unused):
    global LAST_EXEC_NS, LAST_RESULTS
    from concourse.bass_utils import run_bass_kernel_spmd

    to_np = lambda a: np.asarray(a)
    x, adapter, wq, wk, wv, wo = map(to_np, (x, adapter, wq, wk, wv, wo))
    gate, freqs_cos, freqs_sin, mask = map(
        to_np, (gate, freqs_cos, freqs_sin, mask))

    if "nc" not in _NC_CACHE:
        _NC_CACHE["nc"] = build_graph()
    nc = _NC_CACHE["nc"]

    in_maps = _prep_inputs(x, adapter, wq, wk, wv, wo, gate,
                           freqs_cos, freqs_sin, mask)
    res = run_bass_kernel_spmd(
        nc, in_maps, core_ids=list(range(NCORES)), trace=TRACE)
    LAST_EXEC_NS = res.exec_time_ns
    LAST_RESULTS = res
    out = np.empty((B, S, DIM), np.float32)
    for c in range(NCORES):
        g, ci = divmod(c, CPG)
        out[g, :, ci * OC:(ci + 1) * OC] = res.results[c]["out"]
    return out
